# revision 24
# baseline (speedup 1.0000x reference)
"""Trainium2 Bass kernel for nn_BaseVectorQuantizer (VQ codebook module).

Data-parallel over the token axis: B*T = 16384 tokens sharded 8 ways
(2048 tokens/core); codebook (8192x8) and projection weights replicated.

Per-core pipeline (all fp32):
  project_in:  h = relu(x @ W1_in + b1_in); z = h @ W2_in + b2_in   (PE)
  LayerNorm(C=8) token-major via bn_stats + Newton-refined rsqrt     (DVE)
  scores s[t,k] = z.e_k - 0.5*||e_k||^2 via K=9 augmented matmul     (PE)
  argmax: reduce_max + max_index over half-tiles                     (DVE)
  encodings = is_equal(s, max) one-hot, written straight to HBM
  quantized = emb[idx] via indirect-DMA gather
  project_out: r = relu(qz @ W1_out + b1_out); q = r @ W2_out + b2_out (PE)
  LayerNorm(E=512) + write q
"""

import numpy as np

B, T, E = 8, 2048, 512
K, C = 8192, 8
N_CORES = 8
TOK_PER_CORE = (B * T) // N_CORES  # 2048
LN_EPS = 1e-5

_NC_CACHE = {}


def _build_nc(tok_per_core):
    import concourse.bass as bass
    import concourse.bacc as bacc
    import concourse.mybir as mybir
    from concourse.tile import TileContext
    from contextlib import ExitStack

    f32 = mybir.dt.float32
    f32r = mybir.dt.float32r
    i32 = mybir.dt.int32
    u32 = mybir.dt.uint32
    AF = mybir.ActivationFunctionType
    ALU = mybir.AluOpType
    AX = mybir.AxisListType

    P = 128
    n_groups = tok_per_core // P          # token tiles of 128
    n_st = tok_per_core // 512            # supertiles of 512 tokens
    assert n_st * 512 == tok_per_core

    nc = bacc.Bacc()

    x_d = nc.dram_tensor("x", [tok_per_core, E], f32, kind="ExternalInput")
    W1i_d = nc.dram_tensor("W1_in", [E, E], f32, kind="ExternalInput")
    b1i_d = nc.dram_tensor("b1_in", [E], f32, kind="ExternalInput")
    W2i_d = nc.dram_tensor("W2_in", [E, C], f32, kind="ExternalInput")
    b2i_d = nc.dram_tensor("b2_in", [C], f32, kind="ExternalInput")
    gi_d = nc.dram_tensor("ln_in_g", [C], f32, kind="ExternalInput")
    bi_d = nc.dram_tensor("ln_in_b", [C], f32, kind="ExternalInput")
    emb_d = nc.dram_tensor("emb", [K, C], f32, kind="ExternalInput")
    W1o_d = nc.dram_tensor("W1_out", [C, E], f32, kind="ExternalInput")
    b1o_d = nc.dram_tensor("b1_out", [E], f32, kind="ExternalInput")
    W2o_d = nc.dram_tensor("W2_out", [E, E], f32, kind="ExternalInput")
    b2o_d = nc.dram_tensor("b2_out", [E], f32, kind="ExternalInput")
    go_d = nc.dram_tensor("ln_out_g", [E], f32, kind="ExternalInput")
    bo_d = nc.dram_tensor("ln_out_b", [E], f32, kind="ExternalInput")

    q_d = nc.dram_tensor("q", [tok_per_core, E], f32, kind="ExternalOutput")
    idx_d = nc.dram_tensor("idx", [tok_per_core, 1], i32, kind="ExternalOutput")
    enc_d = nc.dram_tensor("enc", [tok_per_core, K], f32, kind="ExternalOutput")

    def bcast_ap(src, p=P):
        # partition-broadcast view of a 1-D DRAM vector
        a = src[:]
        return bass.AP(tensor=a.tensor, offset=a.offset, ap=[[0, p]] + list(a.ap))

    with TileContext(nc) as tc, ExitStack() as ctx:
        singles = ctx.enter_context(tc.tile_pool(name="singles", bufs=1))
        psum = ctx.enter_context(tc.tile_pool(name="psum", bufs=1, space="PSUM"))

        # ---- constants / weights ----
        id_sb = singles.tile([P, P], f32)
        with tc.tile_pool(name="idsetup", bufs=1) as idsetup:
            rampP = idsetup.tile([P, P], f32)
            colP = idsetup.tile([P, 1], f32)
            nc.gpsimd.iota(rampP, pattern=[[1, P]], base=0, channel_multiplier=0,
                           allow_small_or_imprecise_dtypes=True)
            nc.gpsimd.iota(colP, pattern=[[1, 1]], base=0, channel_multiplier=1,
                           allow_small_or_imprecise_dtypes=True)
            nc.vector.tensor_scalar(out=id_sb, in0=rampP, scalar1=colP, scalar2=None,
                                    op0=ALU.is_equal)

        W1sb = singles.tile([P, 4, E], f32)
        nc.sync.dma_start(out=W1sb, in_=W1i_d.rearrange("(kc p) m -> p kc m", p=P))
        W2isb = singles.tile([P, 4, C], f32)
        nc.sync.dma_start(out=W2isb, in_=W2i_d.rearrange("(kc p) m -> p kc m", p=P))
        W2osb = singles.tile([P, 4, E], f32)
        nc.sync.dma_start(out=W2osb, in_=W2o_d.rearrange("(kc p) m -> p kc m", p=P))
        W1o_aug = singles.tile([9, E], f32)
        nc.sync.dma_start(out=W1o_aug[0:8, :], in_=W1o_d[:, :])
        nc.sync.dma_start(out=W1o_aug[8:9, :],
                          in_=b1o_d.rearrange("(one e) -> one e", one=1))
        b1sb = singles.tile([P, 4], f32)
        nc.sync.dma_start(out=b1sb, in_=b1i_d.rearrange("(fc p) -> p fc", p=P))
        b2isb = singles.tile([C, 1], f32)
        nc.sync.dma_start(out=b2isb, in_=b2i_d.rearrange("(c one) -> c one", one=1))
        gi_b = singles.tile([P, C], f32)
        nc.gpsimd.dma_start(out=gi_b, in_=bcast_ap(gi_d))
        bi_b = singles.tile([P, C], f32)
        nc.gpsimd.dma_start(out=bi_b, in_=bcast_ap(bi_d))
        b2ob = singles.tile([P, E], f32)
        nc.gpsimd.dma_start(out=b2ob, in_=bcast_ap(b2o_d))
        gob = singles.tile([P, E], f32)
        nc.gpsimd.dma_start(out=gob, in_=bcast_ap(go_d))
        bob = singles.tile([P, E], f32)
        nc.gpsimd.dma_start(out=bob, in_=bcast_ap(bo_d))
        ones8 = singles.tile([C, 1], f32)
        nc.gpsimd.memset(ones8, 1.0)
        eps_t = singles.tile([P, 1], f32)
        nc.gpsimd.memset(eps_t, LN_EPS)
        zero1 = singles.tile([P, 1], f32)
        nc.gpsimd.memset(zero1, 0.0)

        idx_all = singles.tile([P, n_groups], f32)

        # ---- embT (transposed codebook) + aug row = -0.5*||e||^2 ----
        embTaug = singles.tile([9, K], f32)
        with tc.tile_pool(name="setup", bufs=2) as setup:
            for c16 in range(K // 512):
                emb_sb = setup.tile([P, 4, C], f32, tag="embld")
                src = emb_d[c16 * 512:(c16 + 1) * 512, :]
                nc.gpsimd.dma_start(out=emb_sb,
                                    in_=src.rearrange("(rc p) c -> p rc c", p=P))
                for rc in range(4):
                    tpp = psum.tile([C, P], f32, tag="tp", bufs=3)
                    nc.tensor.transpose(tpp, emb_sb[:, rc, :], id_sb)
                    nc.scalar.activation(
                        out=embTaug[0:8, c16 * 512 + rc * P: c16 * 512 + (rc + 1) * P],
                        in_=tpp, func=AF.Copy)
            esq = setup.tile([C, K], f32, tag="esq")
            nc.vector.tensor_mul(esq, embTaug[0:8, :], embTaug[0:8, :])
            ebias = setup.tile([1, K], f32, tag="ebias")
            for j in range(K // 512):
                sp1 = psum.tile([1, 512], f32, tag="tp", bufs=3)
                nc.tensor.matmul(sp1, lhsT=ones8, rhs=esq[:, j * 512:(j + 1) * 512],
                                 start=True, stop=True)
                nc.vector.tensor_scalar(out=ebias[:, j * 512:(j + 1) * 512],
                                        in0=sp1, scalar1=-0.5, scalar2=None,
                                        op0=ALU.mult)
            nc.sync.dma_start(out=embTaug[8:9, :], in_=ebias)

        # ---- work pools ----
        work = ctx.enter_context(tc.tile_pool(name="work", bufs=1))
        spool = ctx.enter_context(tc.tile_pool(name="spool", bufs=9))
        encp = ctx.enter_context(tc.tile_pool(name="encp", bufs=2))
        small = ctx.enter_context(tc.tile_pool(name="small", bufs=4))

        def rstd_nr(var_ap, width, iters):
            # y ~= 1/sqrt(var+eps), Newton-refined
            vpe = small.tile([P, width], f32, tag="stat")
            nc.vector.tensor_scalar(out=vpe, in0=var_ap, scalar1=LN_EPS,
                                    scalar2=None, op0=ALU.add)
            s0 = small.tile([P, width], f32, tag="stat")
            nc.scalar.activation(out=s0, in_=var_ap, func=AF.Sqrt, bias=eps_t)
            y = small.tile([P, width], f32, tag="stat")
            nc.vector.reciprocal(out=y, in_=s0)
            for _ in range(iters):
                t1 = small.tile([P, width], f32, tag="stat")
                nc.vector.tensor_mul(t1, y, y)
                nc.vector.tensor_mul(t1, t1, vpe)
                nc.vector.tensor_scalar(out=t1, in0=t1, scalar1=-0.5, scalar2=1.5,
                                        op0=ALU.mult, op1=ALU.add)
                nc.vector.tensor_mul(y, y, t1)
            return y

        for st in range(n_st):
            t0 = st * 512
            # -- load x supertile, build xT --
            xa = work.tile([P, 4, E], f32, tag="xa", bufs=1)
            nc.sync.dma_start(
                out=xa, in_=x_d[t0:t0 + 512, :].rearrange("(tt p) e -> p tt e", p=P))
            xT = work.tile([P, 4, 512], f32, tag="xT", bufs=1)
            for tt in range(4):
                for ec in range(4):
                    tp = psum.tile([P, P], f32, tag="tp", bufs=3)
                    nc.tensor.transpose(tp, xa[:, tt, ec * P:(ec + 1) * P], id_sb)
                    nc.scalar.activation(out=xT[:, ec, tt * P:(tt + 1) * P],
                                         in_=tp, func=AF.Copy)
            # -- h^T = relu(W1^T x^T + b1) feature-major --
            hT = work.tile([P, 4, 512], f32, tag="hT", bufs=1)
            for fc in range(4):
                hp = psum.tile([P, 512], f32, tag="mm", bufs=5)
                for ec in range(4):
                    nc.tensor.matmul(hp, lhsT=W1sb[:, ec, fc * P:(fc + 1) * P],
                                     rhs=xT[:, ec, :],
                                     start=(ec == 0), stop=(ec == 3))
                nc.scalar.activation(out=hT[:, fc, :], in_=hp, func=AF.Relu,
                                     bias=b1sb[:, fc:fc + 1])
            # -- z pre-LN feature-major [8, 512] --
            zp = psum.tile([C, 512], f32, tag="tp", bufs=3)
            for fc in range(4):
                nc.tensor.matmul(zp, lhsT=W2isb[:, fc, :], rhs=hT[:, fc, :],
                                 start=(fc == 0), stop=(fc == 3))
            zsb = work.tile([C, 512], f32, tag="zsb", bufs=2)
            nc.scalar.activation(out=zsb, in_=zp, func=AF.Identity, bias=b2isb)
            # -- token-major z + LayerNorm --
            ztn = work.tile([P, 4, C], f32, tag="ztn", bufs=2)
            mean_b = small.tile([P, 4], f32, tag="lnin")
            var_b = small.tile([P, 4], f32, tag="lnin")
            zts = []
            for tt in range(4):
                tpz = psum.tile([P, C], f32, tag="tp", bufs=3)
                nc.tensor.transpose(tpz, zsb[:, tt * P:(tt + 1) * P], id_sb[0:C, 0:C])
                zt = small.tile([P, C], f32, tag="zt", bufs=5)
                nc.scalar.activation(out=zt, in_=tpz, func=AF.Copy)
                stats = small.tile([P, 6], f32, tag="stats")
                nc.vector.bn_stats(out=stats, in_=zt)
                mv = small.tile([P, 2], f32, tag="stats")
                nc.vector.bn_aggr(out=mv, in_=stats)
                nc.vector.tensor_copy(mean_b[:, tt:tt + 1], mv[:, 0:1])
                nc.vector.tensor_copy(var_b[:, tt:tt + 1], mv[:, 1:2])
                zts.append(zt)
            rstd_b = rstd_nr(var_b, 4, iters=2)
            for tt in range(4):
                nc.vector.tensor_scalar(out=ztn[:, tt, :], in0=zts[tt],
                                        scalar1=mean_b[:, tt:tt + 1],
                                        scalar2=rstd_b[:, tt:tt + 1],
                                        op0=ALU.subtract, op1=ALU.mult)
                nc.vector.tensor_mul(ztn[:, tt, :], ztn[:, tt, :], gi_b)
                nc.vector.tensor_add(ztn[:, tt, :], ztn[:, tt, :], bi_b)

            # -- per 128-token group: scores, argmax, enc (software-pipelined:
            #    group tt's argmax tail is traced after group tt+1's score fill
            #    so the scheduler interleaves DVE with PE/ACT) --
            qzTs = []
            NQ = 4
            QW = K // NQ  # 2048
            pend = None  # (g, m-tile, quarts)

            def fill_scores(tt):
                g = st * 4 + tt
                zaT = small.tile([9, P], f32, tag="zaT", name=f"zaT_{g}")
                nc.gpsimd.memset(zaT, 1.0)
                tpzT = psum.tile([C, P], f32, tag="tp", bufs=3, name=f"tpzT_{g}")
                nc.tensor.transpose(tpzT, ztn[:, tt, :], id_sb)
                nc.scalar.activation(out=zaT[0:8, :], in_=tpzT, func=AF.Copy)
                squarts = []
                for qq in range(NQ):
                    sq_t = spool.tile([P, QW], f32, tag="s", name=f"sq_{g}_{qq}")
                    squarts.append(sq_t)
                for j in range(K // 512):
                    sp = psum.tile([P, 512], f32, tag="mm", bufs=5, name=f"sp_{g}_{j}")
                    nc.tensor.matmul(sp, lhsT=zaT,
                                     rhs=embTaug[:, j * 512:(j + 1) * 512],
                                     start=True, stop=True)
                    dst = squarts[j // 4]
                    nc.scalar.activation(out=dst[:, (j % 4) * 512:(j % 4 + 1) * 512],
                                         in_=sp, func=AF.Copy)
                # quarter maxima (can start as quarters land)
                mq = small.tile([P, 4], f32, tag="am4", name=f"mq_{g}")
                for qq in range(NQ):
                    nc.vector.reduce_max(mq[:, qq:qq + 1], squarts[qq], axis=AX.X)
                m = small.tile([P, 1], f32, tag="am", name=f"m_{g}")
                nc.vector.reduce_max(m, mq, axis=AX.X)
                return (g, m, squarts)

            def argmax_tail(state):
                g, m, squarts = state
                gb = g * P
                m8 = small.tile([P, 8], f32, tag="am8", name=f"m8_{g}")
                nc.vector.memset(m8, 3.0e38)
                nc.vector.tensor_copy(m8[:, 0:1], m)
                iqf = small.tile([P, 4], f32, tag="am4", name=f"iqf_{g}")
                for qq in range(NQ):
                    iq = small.tile([P, 8], u32, tag="amidx", name=f"iq_{g}_{qq}")
                    nc.vector.max_index(iq, m8, squarts[qq])
                    nc.vector.tensor_copy(iqf[:, qq:qq + 1], iq[:, 0:1])
                    if qq:
                        nc.vector.tensor_scalar(out=iqf[:, qq:qq + 1],
                                                in0=iqf[:, qq:qq + 1],
                                                scalar1=float(QW * qq),
                                                scalar2=None, op0=ALU.add)
                    # one-hot for this quarter
                    encq = encp.tile([P, QW], f32, tag="enc", bufs=2,
                                     name=f"encq_{g}_{qq}")
                    nc.vector.tensor_scalar(out=encq, in0=squarts[qq],
                                            scalar1=m, scalar2=None, op0=ALU.is_equal)
                    nc.sync.dma_start(
                        out=enc_d[gb:gb + P, qq * QW:(qq + 1) * QW], in_=encq)
                idxf = small.tile([P, 1], f32, tag="am", name=f"idxf_{g}")
                nc.vector.tensor_reduce(idxf, iqf, axis=AX.X, op=ALU.min)
                nc.vector.tensor_copy(idx_all[:, g:g + 1], idxf)
                idxi = small.tile([P, 1], i32, tag="ami", name=f"idxi_{g}")
                nc.vector.tensor_copy(idxi, idxf)
                # quantized = emb[idx] (gather)
                qz = small.tile([P, C], f32, tag="qz", name=f"qz_{g}")
                nc.gpsimd.indirect_dma_start(
                    out=qz, out_offset=None, in_=emb_d[:, :],
                    in_offset=bass.IndirectOffsetOnAxis(ap=idxi, axis=0))
                qzT = small.tile([9, P], f32, tag="qzT", bufs=5, name=f"qzT_{g}")
                nc.gpsimd.memset(qzT, 1.0)
                tpq = psum.tile([C, P], f32, tag="tp", bufs=3, name=f"tpq_{g}")
                nc.tensor.transpose(tpq, qz, id_sb)
                nc.scalar.activation(out=qzT[0:8, :], in_=tpq, func=AF.Copy)
                qzTs.append(qzT)

            for tt in range(4):
                state = fill_scores(tt)
                if pend is not None:
                    argmax_tail(pend)
                pend = state
            argmax_tail(pend)

            # -- batched project_out + LayerNorm for the 4 groups --
            q_sbs = []
            mean4 = small.tile([P, 4], f32, tag="lnout")
            var4 = small.tile([P, 4], f32, tag="lnout")
            for tt in range(4):
                rp = psum.tile([P, 512], f32, tag="mm", bufs=5)
                nc.tensor.matmul(rp, lhsT=qzTs[tt], rhs=W1o_aug, start=True, stop=True)
                r_sb = work.tile([P, E], f32, tag="rsb", bufs=2)
                nc.scalar.activation(out=r_sb, in_=rp, func=AF.Relu, bias=zero1)
                rT = work.tile([P, 4, P], f32, tag="rT", bufs=2)
                for fc in range(4):
                    tpr = psum.tile([P, P], f32, tag="tp", bufs=3)
                    nc.tensor.transpose(tpr, r_sb[:, fc * P:(fc + 1) * P], id_sb)
                    nc.scalar.activation(out=rT[:, fc, :], in_=tpr, func=AF.Copy)
                qp = psum.tile([P, 512], f32, tag="mm", bufs=5)
                for fc in range(4):
                    nc.tensor.matmul(qp, lhsT=rT[:, fc, :], rhs=W2osb[:, fc, :],
                                     start=(fc == 0), stop=(fc == 3))
                q_sb = work.tile([P, E], f32, tag="qsb", bufs=5, name=f"q_sb_{st}_{tt}")
                nc.scalar.activation(out=q_sb, in_=qp, func=AF.Copy)
                nc.gpsimd.tensor_add(q_sb, q_sb, b2ob)
                stats = small.tile([P, 6], f32, tag="stats")
                nc.vector.bn_stats(out=stats, in_=q_sb)
                mv = small.tile([P, 2], f32, tag="stats")
                nc.vector.bn_aggr(out=mv, in_=stats)
                nc.vector.tensor_copy(mean4[:, tt:tt + 1], mv[:, 0:1])
                nc.vector.tensor_copy(var4[:, tt:tt + 1], mv[:, 1:2])
                q_sbs.append(q_sb)
            rstd4 = rstd_nr(var4, 4, iters=1)
            for tt in range(4):
                qn = work.tile([P, E], f32, tag="qn", bufs=2)
                nc.vector.tensor_scalar(out=qn, in0=q_sbs[tt],
                                        scalar1=mean4[:, tt:tt + 1],
                                        scalar2=rstd4[:, tt:tt + 1],
                                        op0=ALU.subtract, op1=ALU.mult)
                nc.gpsimd.tensor_mul(qn, qn, gob)
                nc.gpsimd.tensor_add(qn, qn, bob)
                nc.gpsimd.dma_start(out=q_d[(st * 4 + tt) * P:(st * 4 + tt) * P + P, :],
                                    in_=qn)

        # -- idx output: [128, n_groups] -> transpose -> [n_groups, 128] i32 --
        for gc in range(0, n_groups, P):
            w = min(P, n_groups - gc)
            tpi = psum.tile([P, P], f32, tag="tp", bufs=3)
            nc.tensor.transpose(tpi[0:w, :], idx_all[:, gc:gc + w], id_sb)
            idxT = small.tile([P, P], i32, tag="idxT")
            nc.vector.tensor_copy(idxT[0:w, :], tpi[0:w, :])
            nc.gpsimd.dma_start(
                out=idx_d.rearrange("(gr p) one -> gr (p one)", p=P)[gc:gc + w, :],
                in_=idxT[0:w, :])

    return nc


def _get_nc(tok_per_core=TOK_PER_CORE):
    if tok_per_core not in _NC_CACHE:
        nc = _build_nc(tok_per_core)
        if not nc.is_finalized():
            nc.finalize()
        _NC_CACHE[tok_per_core] = nc
    return _NC_CACHE[tok_per_core]


def kernel(**inputs):
    from concourse.bass_utils import run_bass_kernel_spmd

    nc = _get_nc(TOK_PER_CORE)
    feats = np.ascontiguousarray(np.asarray(inputs["features"], dtype=np.float32))
    N = B * T
    xf = feats.reshape(N, E)
    shared = {}
    for name in ("W1_in", "b1_in", "W2_in", "b2_in", "ln_in_g", "ln_in_b", "emb",
                 "W1_out", "b1_out", "W2_out", "b2_out", "ln_out_g", "ln_out_b"):
        shared[name] = np.ascontiguousarray(np.asarray(inputs[name], dtype=np.float32))
    in_maps = []
    for c in range(N_CORES):
        m = dict(shared)
        m["x"] = np.ascontiguousarray(xf[c * TOK_PER_CORE:(c + 1) * TOK_PER_CORE])
        in_maps.append(m)

    res = run_bass_kernel_spmd(nc, in_maps, list(range(N_CORES))).results

    q = np.concatenate([res[c]["q"] for c in range(N_CORES)], axis=0)
    idx = np.concatenate([res[c]["idx"] for c in range(N_CORES)], axis=0)
    enc = np.concatenate([res[c]["enc"] for c in range(N_CORES)], axis=0)
    return (q.reshape(B, T, E).astype(np.float32),
            idx.astype(np.int32).reshape(N, 1),
            enc.astype(np.float32))
